# revision 4
# baseline (speedup 1.0000x reference)
"""RGCN 2-layer (basis decomposition) on 8 Trainium2 NeuronCores — streaming
gather/scatter design.

N=50000, E=1600000, R=50, B=30, H=16, C=4.

Per core a (src-sharded, NS=6656 nodes):
- table1[(s,t)] = comp1 @ basis1 built on device (fp8 inputs, bf16 rows at
  256B stride); table2[(s,t)] = x[s] @ W2[t] built on device after layer 1
  (f32 rows at 256B stride, first 4 f32 used).
- Edge tokens sorted by (seg, u=dst%2, rank); per seg one dma_gather
  (int16 keys < 32000, <=12288 tokens/call), vector compact of the useful
  bytes, then dma_scatter_add calls (<=4096 tokens) per (u, rank) into
  acc_u[q=dst//2, (rank%4)*16:...] — row indices are unique within every
  call because dma_scatter_add loses updates on duplicate indices; calls
  to the same column are WAW-serialized by the tile scheduler.
- acc_u [1024,1664] f32 = [26624 rows x 64 f32]; ReduceScatter per u;
  epilogue computes x = relu(xsum*invc + root1), then table2 = x@W2,
  second identical stream pass (same idx tensors), RS, log_softmax.

Storage order s_til = (2g+u)*128+p for local node sl=2*(p*26+g)+u matches the
epilogue tile layout so no on-device permutation is ever needed.
"""

import os
import sys

sys.path.insert(0, "/opt/trn_rl_repo")



import threading as _threading

import numpy as np
import ml_dtypes

import concourse.bass as bass
import concourse.bacc as bacc
import concourse.mybir as mybir
import concourse.tile as tile
from concourse.bass_utils import run_bass_kernel_spmd
from concourse.masks import make_identity

N, E, R, B, H, C = 50000, 1600000, 50, 30, 16, 4
NC = 8
NS = 6656                 # nodes per core (52*128)
NP = NC * NS              # 53248
G26 = 26                  # q-groups per partition in epilogue tile
QROWS = NP // 2           # 26624 global acc rows (= 1024*26)
QS = QROWS // NC          # 3328 rows per core slice
DUMP = 26000              # pad-token scatter row (a pad-node row, discarded)
SEG_S = 640               # s_til per gather segment
SEGROWS = SEG_S * R       # 32000 table rows per segment (< 32767)
NSEG = (NS + SEG_S - 1) // SEG_S   # 11
TROWS = NS * R            # 332800
CHUNK_TOK = 12288         # max tokens per gather call (SBUF budget)
SCAT_MAX = 4096           # max tokens per scatter call (HW packet/ring limit)
LAST_RUN_WALL_S = None

F32 = mybir.dt.float32
BF16 = mybir.dt.bfloat16
FP8 = mybir.dt.float8e4
I16 = mybir.dt.int16

try:
    import jax as _jax
    _jax.config.update("jax_compilation_cache_dir", "/tmp/jax_cc_cache")
    _jax.config.update("jax_persistent_cache_min_compile_time_secs", 0.0)
except Exception:
    pass


def build_program(schedule, total_tok):
    """schedule: list of chunks; chunk = (seg, tok_base, ntok, calls) with
    calls = list of (u, r, call_base, call_ntok); token offsets global."""
    nc = bacc.Bacc("TRN2", target_bir_lowering=False, debug=False, num_devices=NC)

    b1s = nc.dram_tensor("b1s", [B, NS * H], FP8, kind="ExternalInput")
    comp1Tb = nc.dram_tensor("comp1Tb", [B, R], FP8, kind="ExternalInput")
    g16 = nc.dram_tensor("g16", [16, total_tok // 16], I16, kind="ExternalInput")
    s16 = nc.dram_tensor("s16", [16, total_tok // 16], I16, kind="ExternalInput")
    gidx = nc.dram_tensor("gidx", [128, total_tok // 16], I16)
    sidx = nc.dram_tensor("sidx", [128, total_tok // 16], I16)
    invc0 = nc.dram_tensor("invc0", [128, G26], F32, kind="ExternalInput")
    invc1 = nc.dram_tensor("invc1", [128, G26], F32, kind="ExternalInput")
    r1g0 = nc.dram_tensor("r1g0", [128, G26 * H], BF16, kind="ExternalInput")
    r1g1 = nc.dram_tensor("r1g1", [128, G26 * H], BF16, kind="ExternalInput")
    comp2T = nc.dram_tensor("comp2T", [B, R], F32, kind="ExternalInput")
    basis2f = nc.dram_tensor("basis2f", [B, H * C], F32, kind="ExternalInput")
    root2 = nc.dram_tensor("root2", [H, C], F32, kind="ExternalInput")
    bias2b = nc.dram_tensor("bias2b", [128, C], F32, kind="ExternalInput")
    outp = nc.dram_tensor("outp", [128, 52 * C], F32, kind="ExternalOutput")

    table1p = nc.dram_tensor("table1p", [TROWS, 128], BF16)
    table2p = nc.dram_tensor("table2p", [TROWS, 64], F32)
    acc = [[nc.dram_tensor(f"acc{l}_{u}", [1024, 26 * 64], F32)
            for u in range(2)] for l in range(2)]
    rso = [[nc.dram_tensor(f"rso{l}_{u}", [128, G26 * 64], F32)
            for u in range(2)] for l in range(2)]

    rg = [list(range(NC))]

    with tile.TileContext(nc) as tc:
        with (
            tc.tile_pool(name="const", bufs=1) as cpool,
            tc.tile_pool(name="work", bufs=2) as wpool,
            tc.tile_pool(name="keep", bufs=1) as kpool,
            tc.tile_pool(name="psum", bufs=1, space="PSUM") as ppool,
        ):
            # ---------- constants ----------
            c2t = cpool.tile([B, R], F32)
            nc.sync.dma_start(out=c2t[:], in_=comp2T[:, :])
            b2f = cpool.tile([B, H * C], F32)
            nc.sync.dma_start(out=b2f[:], in_=basis2f[:, :])
            r2t = cpool.tile([H, C], F32)
            nc.sync.dma_start(out=r2t[:], in_=root2[:, :])
            bb2 = cpool.tile([128, C], F32)
            nc.sync.dma_start(out=bb2[:], in_=bias2b[:, :])
            ic = [cpool.tile([128, G26], F32, name=f"ic{u}", tag=f"ic{u}")
                  for u in range(2)]
            nc.sync.dma_start(out=ic[0][:], in_=invc0[:, :])
            nc.sync.dma_start(out=ic[1][:], in_=invc1[:, :])
            r1 = [cpool.tile([128, G26 * H], BF16, name=f"r1{u}", tag=f"r1{u}")
                  for u in range(2)]
            nc.sync.dma_start(out=r1[0][:], in_=r1g0[:, :])
            nc.sync.dma_start(out=r1[1][:], in_=r1g1[:, :])
            ident = cpool.tile([128, 128], F32)
            make_identity(nc, ident[:])

            # ---------- build table1 = comp1 @ basis1 on device ----------
            c1t = cpool.tile([B, R], FP8)
            nc.sync.dma_start(out=c1t[:], in_=comp1Tb[:, :])
            with (
                tc.tile_pool(name="tb1", bufs=2) as tpool,
                tc.tile_pool(name="psum_tb1", bufs=2, space="PSUM") as tppool,
            ):
                for qc in range(8):          # 832 s_til per chunk
                    bt = tpool.tile([B, 832 * H], FP8, tag="bt")
                    nc.sync.dma_start(
                        out=bt[:],
                        in_=b1s[:, qc * 832 * H : (qc + 1) * 832 * H],
                    )
                    for bb in range(13):     # 64 s_til per DMA (2 matmuls)
                        nblk = 2
                        sb = tpool.tile([R, 64 * 128], BF16, tag="sb")
                        for k in range(nblk):
                            blk = bb * 2 + k
                            psb = tppool.tile([R, 32 * H], F32, tag="psb")
                            nc.tensor.matmul(
                                psb[:], c1t[:],
                                bt[:, blk * 512 : (blk + 1) * 512],
                                start=True, stop=True,
                            )
                            nc.scalar.copy(
                                out=sb[:, k * 4096 :]
                                .rearrange("t (s w) -> t s w", w=128)[
                                    :, 0:32, 0:16],
                                in_=psb[:].rearrange("t (s h) -> t s h", h=H),
                            )
                        s0 = qc * 832 + bb * 64
                        ns_ = nblk * 32
                        nc.sync.dma_start(
                            out=table1p[s0 * R : (s0 + ns_) * R, :]
                            .rearrange("(s t) w -> t s w", t=R),
                            in_=sb[:, : ns_ * 128]
                            .rearrange("t (s w) -> t s w", w=128),
                        )

            # ---------- replicate idx wraps to all 8 gpsimd core groups ----
            for k in range(8):
                nc.sync.dma_start(
                    out=gidx[k * 16 : (k + 1) * 16, :], in_=g16[:, :])
                nc.sync.dma_start(
                    out=sidx[k * 16 : (k + 1) * 16, :], in_=s16[:, :])

            # ---------- zero accumulators ----------
            zt = cpool.tile([128, 26 * 64], F32)
            nc.vector.memset(zt[:], 0.0)
            for l in range(2):
                for u in range(2):
                    for k in range(8):
                        nc.sync.dma_start(
                            out=acc[l][u][k * 128 : (k + 1) * 128, :],
                            in_=zt[:],
                        )

            # ---------- streaming pass (shared for both layers) ----------
            def stream(layer, table, esz, gdt, gpool):
                accl = acc[layer]
                for seg, tok_base, ntok, calls in schedule:
                    rb = seg * SEGROWS
                    re_ = min(rb + SEGROWS, TROWS)
                    git = wpool.tile([128, CHUNK_TOK // 16], I16, tag="git")
                    nc.sync.dma_start(
                        out=git[:, : ntok // 16],
                        in_=gidx[:, tok_base // 16 : (tok_base + ntok) // 16],
                    )
                    sit = wpool.tile([128, CHUNK_TOK // 16], I16, tag="sit")
                    nc.sync.dma_start(
                        out=sit[:, : ntok // 16],
                        in_=sidx[:, tok_base // 16 : (tok_base + ntok) // 16],
                    )
                    width = 256 // mybir.dt.size(gdt)
                    gt = gpool.tile([128, CHUNK_TOK // 128, width], gdt, tag="gt")
                    nc.gpsimd.dma_gather(
                        gt[:, : ntok // 128, :],
                        table[rb:re_, :],
                        git[:, : ntok // 16],
                        ntok, ntok, width, single_packet=False,
                    )
                    ct = gpool.tile([128, CHUNK_TOK // 128, esz], F32, tag="ct")
                    nc.vector.tensor_copy(
                        out=ct[:, : ntok // 128, :],
                        in_=gt[:, : ntok // 128, :esz],
                    )
                    for u, r, cbase, cntok in calls:
                        lo = (cbase - tok_base) // 128
                        hi = lo + cntok // 128
                        nc.gpsimd.dma_scatter_add(
                            accl[u][:, :]
                            .rearrange("a (b c) -> (a b) c", c=64)[
                                :, (r % 4) * 16 : (r % 4) * 16 + esz
                            ],
                            ct[:, lo:hi, :],
                            sit[:, (cbase - tok_base) // 16 :
                                (cbase - tok_base + cntok) // 16],
                            cntok, cntok, esz, elem_step=64,
                            single_packet=False,
                        )

            with tc.tile_pool(name="gath0", bufs=2) as gp0:
                stream(0, table1p, 16, BF16, gp0)

            # ---------- ReduceScatter layer 1 ----------
            for u in range(2):
                nc.gpsimd.collective_compute(
                    "ReduceScatter", mybir.AluOpType.add, replica_groups=rg,
                    ins=[acc[0][u].ap().opt()], outs=[rso[0][u].ap().opt()],
                )

            # ---------- x epilogue ----------
            xv = kpool.tile([128, 52 * H], F32)   # x in [p][m=2g+u][h]
            xv4 = xv[:].rearrange("p (g two h) -> p g two h", two=2, h=H)
            for u in range(2):
                ru = wpool.tile([128, G26 * 64], F32, tag=f"ru{u}")
                nc.sync.dma_start(out=ru[:], in_=rso[0][u][:, :])
                xs = wpool.tile([128, G26 * H], F32, tag=f"xs{u}")
                nc.vector.tensor_reduce(
                    out=xs[:],
                    in_=ru[:].rearrange("p (g r h) -> p g h r", r=4, h=16),
                    axis=mybir.AxisListType.X,
                    op=mybir.AluOpType.add,
                )
                nc.vector.tensor_tensor(
                    out=xs[:].rearrange("p (g h) -> p g h", h=H),
                    in0=xs[:].rearrange("p (g h) -> p g h", h=H),
                    in1=ic[u][:].rearrange("p g -> p g ()").to_broadcast(
                        [128, G26, H]),
                    op=mybir.AluOpType.mult,
                )
                nc.vector.tensor_add(out=xs[:], in0=xs[:], in1=r1[u][:])
                # relu into x view at m=2g+u (stride 2H over g, offset u*H)
                nc.scalar.activation(
                    xv4[:, :, u],
                    xs[:].rearrange("p (g h) -> p g h", h=H),
                    mybir.ActivationFunctionType.Relu,
                )

            # ---------- xT via TensorE transposes ----------
            xT = kpool.tile([H, NS], F32)
            for mb in range(13):
                pst = ppool.tile([H, 512], F32, tag="pst")
                for k in range(4):
                    m = mb * 4 + k
                    nc.tensor.transpose(
                        pst[:, k * 128 : (k + 1) * 128],
                        xv[:, m * H : (m + 1) * H], ident[:])
                nc.scalar.copy(
                    out=xT[:, mb * 512 : (mb + 1) * 512], in_=pst[:])

            # ---------- W2 = comp2 @ basis2 ----------
            w2ps = ppool.tile([H, C, R], F32, tag="w2ps")
            b2v = b2f[:].rearrange("b (h c) -> b h c", c=C)
            for c in range(C):
                nc.tensor.matmul(w2ps[:, c], b2v[:, :, c], c2t[:, :],
                                 start=True, stop=True)
            w2f = cpool.tile([H, R * C], F32)
            nc.scalar.copy(
                out=w2f[:].rearrange("h (t c) -> h t c", c=C),
                in_=w2ps[:].rearrange("h c t -> h t c"),
            )

            # ---------- table2 rows = x @ W2 ----------
            for blk in range(52):
                psm = ppool.tile([128, R * C], F32, tag="psm")
                nc.tensor.matmul(
                    psm[:], xT[:, blk * 128 : (blk + 1) * 128], w2f[:],
                    start=True, stop=True,
                )
                m2sb = wpool.tile([128, R * 64], F32, tag="m2sb")
                nc.scalar.copy(
                    out=m2sb[:].rearrange("p (t w) -> p t w", w=64)[:, :, 0:C],
                    in_=psm[:].rearrange("p (t c) -> p t c", c=C),
                )
                nc.sync.dma_start(
                    out=table2p[blk * 128 * R : (blk + 1) * 128 * R, :]
                    .rearrange("(p t) w -> p t w", t=R),
                    in_=m2sb[:].rearrange("p (t w) -> p t w", w=64),
                )

            # ---------- layer 2 stream ----------
            with tc.tile_pool(name="gath1", bufs=2) as gp1:
                stream(1, table2p, C, F32, gp1)

            # ---------- ReduceScatter layer 2 ----------
            for u in range(2):
                nc.gpsimd.collective_compute(
                    "ReduceScatter", mybir.AluOpType.add, replica_groups=rg,
                    ins=[acc[1][u].ap().opt()], outs=[rso[1][u].ap().opt()],
                )

            # ---------- x @ root2 ----------
            xr2 = kpool.tile([128, 52 * C], F32)
            psr = ppool.tile([128, 52 * C], F32, tag="psr")
            for blk in range(52):
                nc.tensor.matmul(
                    psr[:, blk * C : (blk + 1) * C],
                    xT[:, blk * 128 : (blk + 1) * 128], r2t[:],
                    start=True, stop=True,
                )
            nc.scalar.copy(out=xr2[:], in_=psr[:])

            # ---------- final epilogue ----------
            z = kpool.tile([128, 52 * C], F32)
            z4 = z[:].rearrange("p (g two c) -> p g two c", two=2, c=C)
            xr24 = xr2[:].rearrange("p (g two c) -> p g two c", two=2, c=C)
            for u in range(2):
                ru2 = wpool.tile([128, G26 * 64], F32, tag=f"ru2{u}")
                nc.sync.dma_start(out=ru2[:], in_=rso[1][u][:, :])
                os_ = wpool.tile([128, G26 * C], F32, tag=f"os{u}")
                nc.vector.tensor_reduce(
                    out=os_[:],
                    in_=ru2[:].rearrange("p (g r s) -> p g s r", r=4, s=16)[
                        :, :, :C
                    ],
                    axis=mybir.AxisListType.X,
                    op=mybir.AluOpType.add,
                )
                nc.vector.tensor_tensor(
                    out=os_[:].rearrange("p (g c) -> p g c", c=C),
                    in0=os_[:].rearrange("p (g c) -> p g c", c=C),
                    in1=ic[u][:].rearrange("p g -> p g ()").to_broadcast(
                        [128, G26, C]),
                    op=mybir.AluOpType.mult,
                )
                # z[m=2g+u] = os + xr2[m] + bias2
                nc.vector.tensor_tensor(
                    out=z4[:, :, u],
                    in0=os_[:].rearrange("p (g c) -> p g c", c=C),
                    in1=xr24[:, :, u],
                    op=mybir.AluOpType.add,
                )
                nc.vector.tensor_tensor(
                    out=z4[:, :, u], in0=z4[:, :, u],
                    in1=bb2[:].rearrange("p c -> p () c").to_broadcast(
                        [128, G26, C]),
                    op=mybir.AluOpType.add,
                )
            # log_softmax over c within each m
            m_ = wpool.tile([128, 52], F32, tag="m_")
            nc.vector.tensor_reduce(
                out=m_[:], in_=z[:].rearrange("p (m c) -> p m c", c=C),
                axis=mybir.AxisListType.X, op=mybir.AluOpType.max,
            )
            zm = wpool.tile([128, 52 * C], F32, tag="zm")
            nc.vector.tensor_tensor(
                out=zm[:].rearrange("p (m c) -> p m c", c=C),
                in0=z[:].rearrange("p (m c) -> p m c", c=C),
                in1=m_[:].rearrange("p m -> p m ()").to_broadcast([128, 52, C]),
                op=mybir.AluOpType.subtract,
            )
            ez = wpool.tile([128, 52 * C], F32, tag="ez")
            nc.scalar.activation(ez[:], zm[:], mybir.ActivationFunctionType.Exp)
            ssum = wpool.tile([128, 52], F32, tag="ssum")
            nc.vector.tensor_reduce(
                out=ssum[:], in_=ez[:].rearrange("p (m c) -> p m c", c=C),
                axis=mybir.AxisListType.X, op=mybir.AluOpType.add,
            )
            lse = wpool.tile([128, 52], F32, tag="lse")
            nc.scalar.activation(lse[:], ssum[:], mybir.ActivationFunctionType.Ln)
            ot = wpool.tile([128, 52 * C], F32, tag="ot")
            nc.vector.tensor_tensor(
                out=ot[:].rearrange("p (m c) -> p m c", c=C),
                in0=zm[:].rearrange("p (m c) -> p m c", c=C),
                in1=lse[:].rearrange("p m -> p m ()").to_broadcast([128, 52, C]),
                op=mybir.AluOpType.subtract,
            )
            nc.sync.dma_start(out=outp[:, :], in_=ot[:])

    nc.compile()
    return nc


def kernel(edge_index, edge_type, edge_norm, basis1, comp1, root1, bias1,
           basis2, comp2, root2, bias2):
    import time as _time
    _tp0 = _time.time()
    edge_index = np.asarray(edge_index)
    et = np.asarray(edge_type).astype(np.int64)
    basis1 = np.asarray(basis1, dtype=np.float32)
    comp1 = np.asarray(comp1, dtype=np.float32)
    root1 = np.asarray(root1, dtype=np.float32)
    bias1 = np.asarray(bias1, dtype=np.float32)
    basis2 = np.asarray(basis2, dtype=np.float32)
    comp2 = np.asarray(comp2, dtype=np.float32)
    root2 = np.asarray(root2, dtype=np.float32)
    bias2 = np.asarray(bias2, dtype=np.float32)

    src = edge_index[0].astype(np.int32)
    dst = edge_index[1].astype(np.int32)
    et32 = et.astype(np.int32)

    core = src // NS
    sl = src - core * NS
    u_s = sl & 1
    j_s = sl >> 1
    stil = (2 * (j_s % G26) + u_s) * 128 + (j_s // G26)
    key = stil * R + et32
    seg = stil // SEG_S
    gkey = (key - seg * SEGROWS).astype(np.int16)
    q = (dst >> 1).astype(np.int16)
    u_d = dst & 1

    # rank within (core, seg, dst-node)
    grp = (core * NSEG + seg) * N + dst
    order = np.argsort(grp, kind="stable")
    go = grp[order]
    first = np.ones(E, bool)
    first[1:] = go[1:] != go[:-1]
    run_start = np.maximum.accumulate(np.where(first, np.arange(E, dtype=np.int32), 0))
    rank = np.empty(E, np.int32)
    rank[order] = np.arange(E, dtype=np.int32) - run_start

    MAXR = int(rank.max()) + 1
    # counts per (core, seg, u, r)
    bidx = ((core * NSEG + seg) * 2 + u_d) * MAXR + rank
    counts = np.bincount(bidx, minlength=NC * NSEG * 2 * MAXR).reshape(
        NC, NSEG, 2, MAXR)
    cmax = counts.max(axis=0)                     # [NSEG, 2, MAXR]
    ntok_call = ((cmax + 127) // 128) * 128       # padded per-call sizes

    # schedule: calls grouped into gather chunks (<= CHUNK_TOK) per seg
    schedule = []       # (seg, tok_base, ntok, calls)
    call_base = {}      # (seg,u,r) -> global token base
    tok = 0
    for s in range(NSEG):
        pend = []
        cbase = tok
        for r in range(MAXR):
            for u in range(2):
                n = int(ntok_call[s, u, r])
                if n == 0:
                    continue
                call_base[(s, u, r)] = tok
                # split into scatter-sized pieces (HW per-call desc limit)
                for k in range(0, n, SCAT_MAX):
                    pn = min(SCAT_MAX, n - k)
                    if tok + pn - cbase > CHUNK_TOK and pend:
                        schedule.append((s, cbase, tok - cbase, pend))
                        pend = []
                        cbase = tok
                    pend.append((u, r, tok, pn))
                    tok += pn
        if pend:
            schedule.append((s, cbase, tok - cbase, pend))
    total_tok = ((tok + 127) // 128) * 128
    if total_tok > tok:
        # pad the final chunk so the gidx/sidx tensors are 128-aligned
        s, cb, nt, calls = schedule[-1]
        u, r, cb2, n2 = calls[-1]
        delta = total_tok - tok
        calls[-1] = (u, r, cb2, n2 + delta)
        schedule[-1] = (s, cb, nt + delta, calls)
        ntok_call[s, u, r] += delta
        tok = total_tok

    # per-core token arrays
    gtok = np.zeros((NC, total_tok), np.int16)
    stok = np.full((NC, total_tok), DUMP, np.int16)
    # position of each edge: call_base + within-call index
    base_lut = np.full((NSEG, 2, MAXR), -1, np.int64)
    for (s, u, r), b in call_base.items():
        base_lut[s, u, r] = b
    cb_edge = base_lut[seg, u_d, rank]
    assert (cb_edge >= 0).all()
    # within-call index: edges of the same (core,seg,u,r) get 0..cnt-1
    cgrp = (bidx * NC + core).astype(np.int32)  # == per-(core,seg,u,r) bucket
    order2 = np.argsort(cgrp, kind="stable")
    co = cgrp[order2]
    first2 = np.ones(E, bool)
    first2[1:] = co[1:] != co[:-1]
    rs2 = np.maximum.accumulate(np.where(first2, np.arange(E, dtype=np.int32), 0))
    within = np.empty(E, np.int32)
    within[order2] = np.arange(E, dtype=np.int32) - rs2
    pos = cb_edge + within
    gtok[core, pos] = gkey
    stok[core, pos] = q

    def wrap16(arr):
        # token j -> [j%16, j//16]; device replicates to 128 partitions
        return np.ascontiguousarray(
            arr.reshape(NC, total_tok // 16, 16).transpose(0, 2, 1))

    gidx_w = wrap16(gtok)
    sidx_w = wrap16(stok)

    stil_all = np.arange(NS)
    p_a = stil_all % 128
    m_a = stil_all // 128
    u_a = m_a % 2
    g_a = m_a // 2
    sl_of_stil = 2 * (p_a * G26 + g_a) + u_a      # s_til -> sl
    # epilogue-layout node maps: [p, g] -> local node 2*(p*26+g)+u
    pp, gg = np.meshgrid(np.arange(128), np.arange(G26), indexing="ij")
    nod_u = [2 * (pp * G26 + gg) + u for u in range(2)]   # [128, 26]

    basis1_f8 = np.zeros((B, NP, H), ml_dtypes.float8_e4m3)
    basis1_f8[:, :N, :] = basis1.astype(ml_dtypes.float8_e4m3)
    cnt = np.bincount(dst, minlength=NP).astype(np.float32)
    invc = np.zeros(NP, np.float32)
    real = np.arange(NP) < N
    invc[real] = 1.0 / np.maximum(cnt[real], 1.0)
    root1p = np.zeros((NP, H), np.float32)
    root1p[:N] = root1 + bias1[None, :]

    comp1Tb = np.ascontiguousarray(comp1.T).astype(ml_dtypes.float8_e4m3)
    comp2T = np.ascontiguousarray(comp2.T)
    basis2f = np.ascontiguousarray(basis2.reshape(B, H * C))
    bias2b = np.broadcast_to(bias2, (128, C)).copy()

    _tb0 = _time.time()
    _box = {}

    def _builder():
        _box["nc"] = build_program(schedule, total_tok)

    _th = _threading.Thread(target=_builder)
    _th.start()
    in_maps = []
    for a in range(NC):
        nodes = a * NS + sl_of_stil
        im = {
            "b1s": np.ascontiguousarray(
                basis1_f8[:, nodes, :].reshape(B, NS * H)),
            "comp1Tb": comp1Tb,
            "g16": gidx_w[a],
            "s16": sidx_w[a],
            "comp2T": comp2T, "basis2f": basis2f,
            "root2": root2, "bias2b": bias2b,
        }
        for u in range(2):
            nl = a * NS + nod_u[u]                        # [128, 26] global
            im[f"invc{u}"] = np.ascontiguousarray(invc[nl])
            im[f"r1g{u}"] = np.ascontiguousarray(
                root1p[nl].reshape(128, G26 * H)).astype(ml_dtypes.bfloat16)
        in_maps.append(im)

    host_prep_s = _time.time() - _tp0
    _th.join()
    nc = _box["nc"]
    build_s = _time.time() - _tb0
    print(f"host prep {host_prep_s:.2f}s  build+inmaps {build_s:.2f}s  "
          f"tokens {total_tok} ({total_tok / E * NC:.2f}x) chunks "
          f"{len(schedule)} calls {sum(len(c[3]) for c in schedule)}")

    _t0 = _time.time()
    res = run_bass_kernel_spmd(nc, in_maps, core_ids=list(range(NC)))
    global LAST_RUN_WALL_S
    LAST_RUN_WALL_S = _time.time() - _t0

    # un-permute: out[p, m*4+c] of core a -> node a*NS + 2*(p*26+m//2)+(m%2)
    full = np.zeros((N, C), np.float32)
    mm = np.arange(52)
    node_pm = (2 * (np.arange(128)[:, None] * G26 + (mm // 2)[None, :])
               + (mm % 2)[None, :])               # [128, 52] local node
    for a in range(NC):
        o = res.results[a]["outp"].reshape(128, 52, C)
        nl = a * NS + node_pm
        keep = nl < N
        full[nl[keep]] = o[keep]
    return full
